# revision 9
# baseline (speedup 1.0000x reference)
"""Bass/Trainium2 kernel for nn_DWAMiddleLayer (low-rank MoE weight-assembly layer).

Math (reference):
    U    = pool[:, :1024].reshape(N, DB, R)      # [512, 256, 4]
    V    = pool[:, 1024:2048].reshape(N, R, DA)  # [512, 4, 256]
    bE   = pool[:, 2048:2304]                    # [512, 256]
    h_t  = h_A @ W_base.T
           + sum_r (alpha * (h_A @ V_r.T)) @ U_r          # never materialize W_assembled
           + alpha @ bE + b_base
    y    = h_A + gamma * h_t ; out = LayerNorm(y) * ln_scale + ln_bias

Distribution: data-parallel over batch B=2048 across 8 cores (BS=256 rows each);
pool/W_base/vectors replicated. h_t is computed in transposed space (feature dim
on partitions, batch on the free dim) so that every matmul contraction dim lands
on partitions naturally. All matmul operands are bf16 (the gamma=1e-2 residual
scaling makes matmul rounding error negligible in the output); bf16 data is
produced by SWDGE cast-DMAs or DVE casts, and all layout transposes go through
the XBAR DMA-transpose path (2-byte dtype), keeping the PE free for real
matmuls. The residual + LayerNorm path uses the untransposed fp32 h_A.
"""

import numpy as np

B, N, D_A, D_B, R = 2048, 512, 256, 256, 4
NC_COUNT = 8
BS = B // NC_COUNT  # 256 batch rows per core
P = 128
LN_EPS = 1e-5
POOL_W = D_B * R + R * D_A + D_B  # 2304 used columns of pool_vectors
U_OFF, V_OFF, BE_OFF = 0, D_B * R, D_B * R + R * D_A

_cache = {}


def _build_nc():
    import concourse.mybir as mybir
    import concourse.tile as tile
    from concourse import bacc

    fp32 = mybir.dt.float32
    bf16 = mybir.dt.bfloat16

    nc = bacc.Bacc("TRN2", target_bir_lowering=False)

    # ---- DRAM I/O (per-core shard shapes) ----
    d_hA = nc.dram_tensor("h_A", [BS, D_A], fp32, kind="ExternalInput")
    d_alpha = nc.dram_tensor("alpha", [BS, N], fp32, kind="ExternalInput")
    d_UV = nc.dram_tensor("UVpool", [N, POOL_W], fp32, kind="ExternalInput")
    d_Wb = nc.dram_tensor("W_base", [D_B, D_A], fp32, kind="ExternalInput")
    d_bb = nc.dram_tensor("b_base", [D_B], fp32, kind="ExternalInput")
    d_gamma = nc.dram_tensor("gamma", [1, 1], fp32, kind="ExternalInput")
    d_lsc = nc.dram_tensor("ln_scale", [D_A], fp32, kind="ExternalInput")
    d_lbi = nc.dram_tensor("ln_bias", [D_A], fp32, kind="ExternalInput")
    d_out = nc.dram_tensor("out", [BS, D_A], fp32, kind="ExternalOutput")

    with tile.TileContext(nc) as tc:
        with (
            tc.tile_pool(name="persist", bufs=1) as persist,
            tc.tile_pool(name="stage", bufs=3) as stage,
            tc.tile_pool(name="sm", bufs=3) as sm,
            tc.tile_pool(name="pp_t", bufs=4, space="PSUM") as pp_t,
            tc.tile_pool(name="pp_acc", bufs=1, space="PSUM") as pp_acc,
        ):
            # ---------- tiny constants ----------
            eps_col = persist.tile([P, 1], fp32)
            nc.vector.memset(eps_col, LN_EPS)
            ones_row = persist.tile([1, BS], bf16)
            nc.vector.memset(ones_row, 1.0)
            # warm the ACT Sqrt table so the LN tail doesn't pay ACT_TABLE_LOAD
            warm = sm.tile([P, 1], fp32, tag="warm")
            nc.scalar.activation(
                warm, eps_col, mybir.ActivationFunctionType.Sqrt, bias=eps_col
            )

            # ---------- loads ----------
            # HWDGE (sync) queue: fp32 loads that start while the gpsimd engine
            # is still in its preamble; DVE casts to bf16.
            hA_sb = persist.tile([P, 2, D_A], fp32)  # [p, b_chunk, a]
            nc.sync.dma_start(hA_sb, d_hA[:].rearrange("(o p) a -> p o a", p=P))
            alpha_f = sm.tile([P, 2, N], fp32, tag="alf")
            nc.sync.dma_start(alpha_f, d_alpha[:].rearrange("(o p) n -> p o n", p=P))
            UVc0_f = stage.tile([P, POOL_W], fp32, tag="uvf")
            nc.sync.dma_start(UVc0_f, d_UV[0:P, :])
            Wb_f = sm.tile([P, 2, D_A], fp32, tag="wbf")
            nc.sync.dma_start(Wb_f, d_Wb[:].rearrange("(o p) a -> p o a", p=P))
            bb_row = persist.tile([1, D_B], fp32)
            nc.sync.dma_start(bb_row, d_bb[:].unsqueeze(0))

            hA_bf = persist.tile([P, 2, D_A], bf16)
            nc.vector.tensor_copy(hA_bf, hA_sb)
            alpha_bf = persist.tile([P, 2, N], bf16)
            nc.vector.tensor_copy(alpha_bf, alpha_f)
            Wb_bf = persist.tile([P, 2, D_A], bf16)
            nc.vector.tensor_copy(Wb_bf, Wb_f)
            bb_bf = persist.tile([1, D_B], bf16)
            nc.vector.tensor_copy(bb_bf, bb_row)

            # pool chunks 1..3 via SWDGE cast-DMA (fp32 HBM read -> bf16 SBUF)
            UVc = [
                stage.tile([P, POOL_W], bf16, tag="uvc", name=f"UVc{o}")
                for o in range(4)
            ]
            nc.vector.tensor_copy(UVc[0], UVc0_f)
            nc.gpsimd.dma_start(UVc[1], d_UV[1 * P : 2 * P, :])
            nc.gpsimd.dma_start(UVc[2], d_UV[2 * P : 3 * P, :])
            nc.gpsimd.dma_start(UVc[3], d_UV[3 * P : 4 * P, :])
            # epilogue-only constants (SWDGE broadcasts, needed late)
            lsc_row = persist.tile([P, D_A], fp32)
            nc.gpsimd.dma_start(lsc_row, d_lsc[:].partition_broadcast(P))
            lbi_row = persist.tile([P, D_A], fp32)
            nc.gpsimd.dma_start(lbi_row, d_lbi[:].partition_broadcast(P))
            gamma_col = persist.tile([P, 1], fp32)
            nc.gpsimd.dma_start(gamma_col, d_gamma[:].to_broadcast([P, 1]))

            # ---------- transposes via XBAR DMA-transpose (bf16, HWDGE) ----------
            def xbar_t(dst, src):
                nc.sync.dma_start(dst, src, transpose=True)

            hAT_b = persist.tile([P, 2, BS], bf16)  # [p_a, a_chunk, b]
            for ach in range(2):
                for bch in range(2):
                    xbar_t(
                        hAT_b[:, ach, bch * P : (bch + 1) * P],
                        hA_bf[:, bch, ach * P : (ach + 1) * P],
                    )
            alphaT_b = persist.tile([P, 4, BS], bf16)  # [p_n, n_chunk, b]
            for och in range(4):
                for bch in range(2):
                    xbar_t(
                        alphaT_b[:, och, bch * P : (bch + 1) * P],
                        alpha_bf[:, bch, och * P : (och + 1) * P],
                    )
            WbT_b = persist.tile([P, 2, D_B], bf16)  # [p_a, a_chunk, c]
            for ach in range(2):
                for cch in range(2):
                    xbar_t(
                        WbT_b[:, ach, cch * P : (cch + 1) * P],
                        Wb_bf[:, cch, ach * P : (ach + 1) * P],
                    )

            # ---------- h_t^T accumulator: 2 psum tiles [c_half, b] ----------
            htT = [
                pp_acc.tile([P, BS], fp32, tag=f"acc{ch}", name=f"htT{ch}")
                for ch in range(2)
            ]
            started = [False, False]

            def acc_mm(ch, lhsT, rhs, last=False):
                nc.tensor.matmul(
                    htT[ch],
                    lhsT=lhsT,
                    rhs=rhs,
                    start=(not started[ch]),
                    stop=last,
                    skip_group_check=True,
                )
                started[ch] = True

            # ---------- main pipeline over expert chunks (o = n//128) ----------
            # V layout per pool row: f = V_OFF + r*256 + a  (r-major)
            # U layout per pool row: f = c*4 + r            (c-major)
            VT_b = persist.tile([P, 2, 2048], bf16)  # [p_a, a_chunk, r*512+o*128+pn]
            U_bfr = persist.tile([P, 4, R, D_B], bf16)  # [p_n, o, r, c]

            for o in range(4):
                # transpose V chunk via XBAR: blocks (r, a_half) of [128n x 128a]
                for r in range(4):
                    for ach in range(2):
                        xbar_t(
                            VT_b[:, ach, r * 512 + o * P : r * 512 + (o + 1) * P],
                            UVc[o][
                                :,
                                V_OFF
                                + r * D_A
                                + ach * P : V_OFF
                                + r * D_A
                                + (ach + 1) * P,
                            ],
                        )

                # destride U chunk (c r) -> (r c) in bf16 on DVE
                nc.vector.tensor_copy(
                    U_bfr[:, o],
                    UVc[o][:, U_OFF : U_OFF + D_B * R].rearrange(
                        "p (c r) -> p r c", r=R
                    ),
                )

                for r in range(4):
                    # mm1: t_r^T[n_chunk, b] = V_r @ h_A^T (contract a)
                    t_ps = pp_t.tile([P, BS], fp32, tag="t")
                    for ach in range(2):
                        nc.tensor.matmul(
                            t_ps,
                            lhsT=VT_b[:, ach, r * 512 + o * P : r * 512 + (o + 1) * P],
                            rhs=hAT_b[:, ach],
                            start=(ach == 0),
                            stop=(ach == 1),
                        )
                    # s_r^T = alpha^T * t_r^T  (evict psum -> bf16 sbuf)
                    s_bf = sm.tile([P, BS], bf16, tag="s")
                    nc.vector.tensor_mul(s_bf, t_ps, alphaT_b[:, o])
                    # mm2: h_t^T += U_r^T-chunks @ s_r^T (contract n)
                    for ch in range(2):
                        acc_mm(ch, U_bfr[:, o, r, ch * P : (ch + 1) * P], s_bf)

                # bias-mm for this chunk: h_t^T += biasE^T @ alpha^T (contract n)
                bE_o = UVc[o][:, BE_OFF : BE_OFF + D_B]
                for ch in range(2):
                    acc_mm(
                        ch, bE_o[:, ch * P : (ch + 1) * P], alphaT_b[:, o], last=(o == 3)
                    )

                if o == 0:
                    # base-mm + b_base rank-1, folded in early (no DMA deps left)
                    for ch in range(2):
                        for ach in range(2):
                            acc_mm(
                                ch, WbT_b[:, ach, ch * P : (ch + 1) * P], hAT_b[:, ach]
                            )
                        acc_mm(ch, bb_bf[:, ch * P : (ch + 1) * P], ones_row)

            # ---------- epilogue: transpose h_t back, residual + LayerNorm in fp32 ----------
            htT_bf = sm.tile([P, 2, BS], bf16, tag="htTbf")
            for ch in range(2):
                nc.any.tensor_copy(htT_bf[:, ch], htT[ch])

            ht_sb = sm.tile([P, 2, D_A], bf16, tag="htsb")  # [p_b, b_chunk, j]
            for bch in range(2):
                for jch in range(2):
                    xbar_t(
                        ht_sb[:, bch, jch * P : (jch + 1) * P],
                        htT_bf[:, jch, bch * P : (bch + 1) * P],
                    )

            out_sb = sm.tile([P, 2, D_A], fp32, tag="out")
            for bch in range(2):
                # y = h_A + gamma * h_t   (fp32 residual)
                y_sb = sm.tile([P, D_A], fp32, tag="y")
                nc.vector.scalar_tensor_tensor(
                    y_sb,
                    in0=ht_sb[:, bch],
                    scalar=gamma_col,
                    in1=hA_sb[:, bch],
                    op0=mybir.AluOpType.mult,
                    op1=mybir.AluOpType.add,
                )
                stats = sm.tile([P, 6], fp32, tag="st")
                nc.vector.bn_stats(stats, y_sb)
                mv = sm.tile([P, 2], fp32, tag="mv")
                nc.vector.bn_aggr(mv, stats)
                # rstd = 1/sqrt(var + eps)
                rstd = sm.tile([P, 1], fp32, tag="rstd")
                nc.scalar.activation(
                    rstd, mv[:, 1:2], mybir.ActivationFunctionType.Sqrt, bias=eps_col
                )
                nc.vector.reciprocal(rstd, rstd)
                # (y - mu) * rstd
                nc.vector.tensor_scalar(
                    out_sb[:, bch],
                    y_sb,
                    scalar1=mv[:, 0:1],
                    scalar2=rstd,
                    op0=mybir.AluOpType.subtract,
                    op1=mybir.AluOpType.mult,
                )
                # * ln_scale + ln_bias
                nc.vector.tensor_mul(out_sb[:, bch], out_sb[:, bch], lsc_row)
                nc.vector.tensor_add(out_sb[:, bch], out_sb[:, bch], lbi_row)

            nc.sync.dma_start(d_out[:].rearrange("(o p) c -> p o c", p=P), out_sb)

    nc.compile()
    return nc


def _get_nc():
    if "nc" not in _cache:
        _cache["nc"] = _build_nc()
    return _cache["nc"]


def make_in_maps(**inputs):
    """Shard full inputs into 8 per-core input maps."""
    f32 = lambda x: np.ascontiguousarray(np.asarray(x), dtype=np.float32)
    h_A = f32(inputs["h_A"])
    alpha = f32(inputs["alpha"])
    pool = np.asarray(inputs["pool_vectors"], dtype=np.float32)
    UVpool = np.ascontiguousarray(pool[:, :POOL_W])
    W_base = f32(inputs["W_base"])
    b_base = f32(inputs["b_base"]).reshape(D_B)
    gamma = f32(inputs["gamma"]).reshape(1, 1)
    ln_scale = f32(inputs["ln_scale"]).reshape(D_A)
    ln_bias = f32(inputs["ln_bias"]).reshape(D_A)

    in_maps = []
    for i in range(NC_COUNT):
        sl = slice(i * BS, (i + 1) * BS)
        in_maps.append(
            {
                "h_A": np.ascontiguousarray(h_A[sl]),
                "alpha": np.ascontiguousarray(alpha[sl]),
                "UVpool": UVpool,
                "W_base": W_base,
                "b_base": b_base,
                "gamma": gamma,
                "ln_scale": ln_scale,
                "ln_bias": ln_bias,
            }
        )
    return in_maps


def run_kernel(trace=False, **inputs):
    from concourse.bass_utils import run_bass_kernel_spmd

    nc = _get_nc()
    in_maps = make_in_maps(**inputs)
    res = run_bass_kernel_spmd(nc, in_maps, core_ids=list(range(NC_COUNT)), trace=trace)
    out = np.concatenate([r["out"] for r in res.results], axis=0)
    return out.astype(np.float32), res


def kernel(**inputs) -> np.ndarray:
    out, _ = run_kernel(trace=False, **inputs)
    return out


# revision 10
# speedup vs baseline: 2.3732x; 2.3732x over previous
"""Bass/Trainium2 kernel for nn_DWAMiddleLayer (low-rank MoE weight-assembly layer).

Math (reference):
    U    = pool[:, :1024].reshape(N, DB, R)      # [512, 256, 4]
    V    = pool[:, 1024:2048].reshape(N, R, DA)  # [512, 4, 256]
    bE   = pool[:, 2048:2304]                    # [512, 256]
    h_t  = h_A @ W_base.T
           + sum_r (alpha * (h_A @ V_r.T)) @ U_r          # never materialize W_assembled
           + alpha @ bE + b_base
    y    = h_A + gamma * h_t ; out = LayerNorm(y) * ln_scale + ln_bias

Distribution: data-parallel over batch B=2048 across 8 cores (BS=256 rows each);
pool/W_base/vectors replicated. h_t is computed in transposed space (feature dim
on partitions, batch on the free dim) so that every matmul contraction dim lands
on partitions naturally; layout transposes are PE identity-matmuls in bf16.
All matmul operands are bf16 (the gamma=1e-2 residual scaling makes matmul
rounding error negligible in the output); bf16 operands come from SWDGE
cast-DMAs (pool chunks) or DVE casts (small tensors, loaded fp32 on the HWDGE
queue which starts earlier). The residual + LayerNorm path uses the
untransposed fp32 h_A directly.
"""

import numpy as np

B, N, D_A, D_B, R = 2048, 512, 256, 256, 4
NC_COUNT = 8
BS = B // NC_COUNT  # 256 batch rows per core
P = 128
LN_EPS = 1e-5
POOL_W = D_B * R + R * D_A + D_B  # 2304 used columns of pool_vectors
U_W = D_B * R  # 1024
VBE_W = R * D_A + D_B  # 1280 (V then biasE)
BE_OFF = R * D_A  # within the V|bE slab

_cache = {}


def _build_nc():
    import concourse.mybir as mybir
    import concourse.tile as tile
    from concourse import bacc

    fp32 = mybir.dt.float32
    bf16 = mybir.dt.bfloat16

    nc = bacc.Bacc("TRN2", target_bir_lowering=False)

    # ---- DRAM I/O (per-core shard shapes) ----
    d_hA = nc.dram_tensor("h_A", [BS, D_A], fp32, kind="ExternalInput")
    d_alpha = nc.dram_tensor("alpha", [BS, N], fp32, kind="ExternalInput")
    d_U = nc.dram_tensor("Upool", [N, U_W], fp32, kind="ExternalInput")
    d_VbE = nc.dram_tensor("VbEpool", [N, VBE_W], fp32, kind="ExternalInput")
    d_Wb = nc.dram_tensor("W_base", [D_B, D_A], fp32, kind="ExternalInput")
    d_bb = nc.dram_tensor("b_base", [D_B], fp32, kind="ExternalInput")
    d_gamma = nc.dram_tensor("gamma", [1, 1], fp32, kind="ExternalInput")
    d_lsc = nc.dram_tensor("ln_scale", [D_A], fp32, kind="ExternalInput")
    d_lbi = nc.dram_tensor("ln_bias", [D_A], fp32, kind="ExternalInput")
    d_id = nc.dram_tensor("ident", [P, P], bf16, kind="ExternalInput")
    d_out = nc.dram_tensor("out", [BS, D_A], fp32, kind="ExternalOutput")

    with tile.TileContext(nc) as tc:
        with (
            tc.tile_pool(name="persist", bufs=1) as persist,
            tc.tile_pool(name="stage", bufs=3) as stage,
            tc.tile_pool(name="sm", bufs=3) as sm,
            tc.tile_pool(name="pp_tr", bufs=3, space="PSUM") as pp_tr,
            tc.tile_pool(name="pp_t", bufs=2, space="PSUM") as pp_t,
            tc.tile_pool(name="pp_acc", bufs=1, space="PSUM") as pp_acc,
        ):
            # ---------- tiny constants ----------
            eps_col = persist.tile([P, 1], fp32)
            nc.vector.memset(eps_col, LN_EPS)
            ones_row = persist.tile([1, BS], bf16)
            nc.vector.memset(ones_row, 1.0)
            # warm the ACT Sqrt table so the LN tail doesn't pay ACT_TABLE_LOAD
            warm = sm.tile([P, 1], fp32, tag="warm")
            nc.scalar.activation(
                warm, eps_col, mybir.ActivationFunctionType.Sqrt, bias=eps_col
            )

            # ---------- HWDGE loads (start while gpsimd is still in preamble) ----------
            ident_b = persist.tile([P, P], bf16)
            nc.sync.dma_start(ident_b, d_id[:])
            hA_sb = persist.tile([P, 2, D_A], fp32)  # [p, b_chunk, a]
            nc.sync.dma_start(hA_sb, d_hA[:].rearrange("(o p) a -> p o a", p=P))
            alpha_f = sm.tile([P, 2, N], fp32, tag="alf")
            nc.sync.dma_start(alpha_f, d_alpha[:].rearrange("(o p) n -> p o n", p=P))
            Wb_f = sm.tile([P, 2, D_A], fp32, tag="wbf")
            nc.sync.dma_start(Wb_f, d_Wb[:].rearrange("(o p) a -> p o a", p=P))
            bb_row = persist.tile([1, D_B], fp32)
            nc.sync.dma_start(bb_row, d_bb[:].unsqueeze(0))
            # chunk 0 of the V|bE slab as fp32 on HWDGE + DVE cast (early start)
            VbE0_f = stage.tile([P, VBE_W], fp32, tag="vbef")
            nc.sync.dma_start(VbE0_f, d_VbE[0:P, :])

            hA_bf = persist.tile([P, 2, D_A], bf16)
            nc.vector.tensor_copy(hA_bf, hA_sb)
            alpha_bf = persist.tile([P, 2, N], bf16)
            nc.vector.tensor_copy(alpha_bf, alpha_f)
            Wb_bf = persist.tile([P, 2, D_A], bf16)
            nc.vector.tensor_copy(Wb_bf, Wb_f)
            bb_bf = persist.tile([1, D_B], bf16)
            nc.vector.tensor_copy(bb_bf, bb_row)

            # ---------- SWDGE cast-DMA stream for the pool ----------
            VbE = [
                stage.tile([P, VBE_W], bf16, tag="vbe", name=f"VbE{o}")
                for o in range(4)
            ]
            U_bf = [
                stage.tile([P, U_W], bf16, tag="ubf", name=f"Ubf{o}") for o in range(4)
            ]
            nc.vector.tensor_copy(VbE[0], VbE0_f)
            nc.gpsimd.dma_start(U_bf[0], d_U[0:P, :])
            nc.gpsimd.dma_start(VbE[1], d_VbE[1 * P : 2 * P, :])
            nc.gpsimd.dma_start(U_bf[1], d_U[1 * P : 2 * P, :])
            nc.gpsimd.dma_start(VbE[2], d_VbE[2 * P : 3 * P, :])
            nc.gpsimd.dma_start(U_bf[2], d_U[2 * P : 3 * P, :])
            nc.gpsimd.dma_start(VbE[3], d_VbE[3 * P : 4 * P, :])
            nc.gpsimd.dma_start(U_bf[3], d_U[3 * P : 4 * P, :])
            # epilogue-only constants (SWDGE broadcasts, needed late)
            lsc_row = persist.tile([P, D_A], fp32)
            nc.gpsimd.dma_start(lsc_row, d_lsc[:].partition_broadcast(P))
            lbi_row = persist.tile([P, D_A], fp32)
            nc.gpsimd.dma_start(lbi_row, d_lbi[:].partition_broadcast(P))
            gamma_col = persist.tile([P, 1], fp32)
            nc.gpsimd.dma_start(gamma_col, d_gamma[:].to_broadcast([P, 1]))

            # ---------- transposes of small operands (PE identity-matmul, bf16) ----------
            hAT_b = persist.tile([P, 2, BS], bf16)  # [p_a, a_chunk, b]
            for ach in range(2):
                ps = pp_tr.tile([P, 512], fp32, tag="tr")
                for bch in range(2):
                    nc.tensor.matmul(
                        ps[:, bch * P : (bch + 1) * P],
                        lhsT=hA_bf[:, bch, ach * P : (ach + 1) * P],
                        rhs=ident_b,
                        start=True,
                        stop=True,
                    )
                nc.any.tensor_copy(hAT_b[:, ach], ps[:, :BS])

            # alpha^T -> bf16 [p_n, n_chunk, b]
            alphaT_b = persist.tile([P, 4, BS], bf16)
            for och in range(4):
                ps = pp_tr.tile([P, 512], fp32, tag="tr")
                for bch in range(2):
                    nc.tensor.matmul(
                        ps[:, bch * P : (bch + 1) * P],
                        lhsT=alpha_bf[:, bch, och * P : (och + 1) * P],
                        rhs=ident_b,
                        start=True,
                        stop=True,
                    )
                nc.any.tensor_copy(alphaT_b[:, och], ps[:, :BS])

            # W_base^T -> bf16 [p_a, a_chunk, c]
            WbT_b = persist.tile([P, 2, D_B], bf16)
            for ach in range(2):
                ps = pp_tr.tile([P, 512], fp32, tag="tr")
                for cch in range(2):
                    nc.tensor.matmul(
                        ps[:, cch * P : (cch + 1) * P],
                        lhsT=Wb_bf[:, cch, ach * P : (ach + 1) * P],
                        rhs=ident_b,
                        start=True,
                        stop=True,
                    )
                nc.any.tensor_copy(WbT_b[:, ach], ps[:, :D_B])

            # ---------- h_t^T accumulator: 2 psum tiles [c_half, b] ----------
            htT = [
                pp_acc.tile([P, BS], fp32, tag=f"acc{ch}", name=f"htT{ch}")
                for ch in range(2)
            ]
            started = [False, False]

            def acc_mm(ch, lhsT, rhs, last=False):
                nc.tensor.matmul(
                    htT[ch],
                    lhsT=lhsT,
                    rhs=rhs,
                    start=(not started[ch]),
                    stop=last,
                    skip_group_check=True,
                )
                started[ch] = True

            # ---------- main pipeline over expert chunks (o = n//128) ----------
            # V layout per VbE row: f = r*256 + a  (r-major); bE at f = 1024..1280
            # U layout per U row:   f = c*4 + r    (c-major)
            VT_b = persist.tile([P, 2, 2048], bf16)  # [p_a, a_chunk, r*512+o*128+pn]
            U_bfr = persist.tile([P, 4, R, D_B], bf16)  # [p_n, o, r, c]

            for o in range(4):
                # transpose V chunk: blocks (r, a_half) of [128n x 128a]
                for ach in range(2):
                    ps = pp_tr.tile([P, 512], fp32, tag="tr")
                    for r in range(4):
                        nc.tensor.matmul(
                            ps[:, r * P : (r + 1) * P],
                            lhsT=VbE[o][:, r * D_A + ach * P : r * D_A + (ach + 1) * P],
                            rhs=ident_b,
                            start=True,
                            stop=True,
                        )
                    # scatter the 4 r-blocks into VT at [r*512 + o*128]
                    dst = VT_b[:, ach].rearrange("p (r q) -> p r q", r=4)[
                        :, :, o * P : (o + 1) * P
                    ]
                    nc.any.tensor_copy(dst, ps[:].rearrange("p (r q) -> p r q", r=4))

                # destride U chunk (c r) -> (r c) in bf16 on DVE
                nc.vector.tensor_copy(
                    U_bfr[:, o],
                    U_bf[o][:].rearrange("p (c r) -> p r c", r=R),
                )

                for r in range(4):
                    # mm1: t_r^T[n_chunk, b] = V_r @ h_A^T (contract a)
                    t_ps = pp_t.tile([P, BS], fp32, tag="t")
                    for ach in range(2):
                        nc.tensor.matmul(
                            t_ps,
                            lhsT=VT_b[:, ach, r * 512 + o * P : r * 512 + (o + 1) * P],
                            rhs=hAT_b[:, ach],
                            start=(ach == 0),
                            stop=(ach == 1),
                        )
                    # s_r^T = alpha^T * t_r^T  (evict psum -> bf16 sbuf)
                    s_bf = sm.tile([P, BS], bf16, tag="s")
                    nc.vector.tensor_mul(s_bf, t_ps, alphaT_b[:, o])
                    # mm2: h_t^T += U_r^T-chunks @ s_r^T (contract n)
                    for ch in range(2):
                        acc_mm(ch, U_bfr[:, o, r, ch * P : (ch + 1) * P], s_bf)

                # bias-mm for this chunk: h_t^T += biasE^T @ alpha^T (contract n)
                bE_o = VbE[o][:, BE_OFF : BE_OFF + D_B]
                for ch in range(2):
                    acc_mm(
                        ch, bE_o[:, ch * P : (ch + 1) * P], alphaT_b[:, o], last=(o == 3)
                    )

                if o == 0:
                    # base-mm + b_base rank-1, folded in early (no DMA deps left)
                    for ch in range(2):
                        for ach in range(2):
                            acc_mm(
                                ch, WbT_b[:, ach, ch * P : (ch + 1) * P], hAT_b[:, ach]
                            )
                        acc_mm(ch, bb_bf[:, ch * P : (ch + 1) * P], ones_row)

            # ---------- epilogue: transpose h_t back, residual + LayerNorm in fp32 ----------
            htT_bf = sm.tile([P, 2, BS], bf16, tag="htTbf")
            for ch in range(2):
                nc.any.tensor_copy(htT_bf[:, ch], htT[ch])

            ht_ps = pp_tr.tile([P, 512], fp32, tag="tr", name="ht_ps")
            for bch in range(2):
                for jch in range(2):
                    nc.tensor.matmul(
                        ht_ps[:, bch * 256 + jch * P : bch * 256 + (jch + 1) * P],
                        lhsT=htT_bf[:, jch, bch * P : (bch + 1) * P],
                        rhs=ident_b,
                        start=True,
                        stop=True,
                        skip_group_check=True,
                    )

            out_sb = sm.tile([P, 2, D_A], fp32, tag="out")
            for bch in range(2):
                # y = h_A + gamma * h_t   (fp32 residual)
                y_sb = sm.tile([P, D_A], fp32, tag="y")
                nc.vector.scalar_tensor_tensor(
                    y_sb,
                    in0=ht_ps[:, bch * 256 : bch * 256 + D_A],
                    scalar=gamma_col,
                    in1=hA_sb[:, bch],
                    op0=mybir.AluOpType.mult,
                    op1=mybir.AluOpType.add,
                )
                stats = sm.tile([P, 6], fp32, tag="st")
                nc.vector.bn_stats(stats, y_sb)
                mv = sm.tile([P, 2], fp32, tag="mv")
                nc.vector.bn_aggr(mv, stats)
                # rstd = 1/sqrt(var + eps)
                rstd = sm.tile([P, 1], fp32, tag="rstd")
                nc.scalar.activation(
                    rstd, mv[:, 1:2], mybir.ActivationFunctionType.Sqrt, bias=eps_col
                )
                nc.vector.reciprocal(rstd, rstd)
                # (y - mu) * rstd
                nc.vector.tensor_scalar(
                    out_sb[:, bch],
                    y_sb,
                    scalar1=mv[:, 0:1],
                    scalar2=rstd,
                    op0=mybir.AluOpType.subtract,
                    op1=mybir.AluOpType.mult,
                )
                # * ln_scale + ln_bias
                nc.vector.tensor_mul(out_sb[:, bch], out_sb[:, bch], lsc_row)
                nc.vector.tensor_add(out_sb[:, bch], out_sb[:, bch], lbi_row)

            nc.sync.dma_start(d_out[:].rearrange("(o p) c -> p o c", p=P), out_sb)

    nc.compile()
    return nc


def _get_nc():
    if "nc" not in _cache:
        _cache["nc"] = _build_nc()
    return _cache["nc"]


def make_in_maps(**inputs):
    """Shard full inputs into 8 per-core input maps."""
    import ml_dtypes

    f32 = lambda x: np.ascontiguousarray(np.asarray(x), dtype=np.float32)
    h_A = f32(inputs["h_A"])
    alpha = f32(inputs["alpha"])
    pool = np.asarray(inputs["pool_vectors"], dtype=np.float32)
    Upool = np.ascontiguousarray(pool[:, :U_W])
    VbEpool = np.ascontiguousarray(pool[:, U_W : U_W + VBE_W])
    W_base = f32(inputs["W_base"])
    b_base = f32(inputs["b_base"]).reshape(D_B)
    gamma = f32(inputs["gamma"]).reshape(1, 1)
    ln_scale = f32(inputs["ln_scale"]).reshape(D_A)
    ln_bias = f32(inputs["ln_bias"]).reshape(D_A)
    ident = np.eye(P, dtype=np.float32).astype(ml_dtypes.bfloat16)

    in_maps = []
    for i in range(NC_COUNT):
        sl = slice(i * BS, (i + 1) * BS)
        in_maps.append(
            {
                "h_A": np.ascontiguousarray(h_A[sl]),
                "alpha": np.ascontiguousarray(alpha[sl]),
                "Upool": Upool,
                "VbEpool": VbEpool,
                "W_base": W_base,
                "b_base": b_base,
                "gamma": gamma,
                "ln_scale": ln_scale,
                "ln_bias": ln_bias,
                "ident": ident,
            }
        )
    return in_maps


def run_kernel(trace=False, **inputs):
    from concourse.bass_utils import run_bass_kernel_spmd

    nc = _get_nc()
    in_maps = make_in_maps(**inputs)
    res = run_bass_kernel_spmd(nc, in_maps, core_ids=list(range(NC_COUNT)), trace=trace)
    out = np.concatenate([r["out"] for r in res.results], axis=0)
    return out.astype(np.float32), res


def kernel(**inputs) -> np.ndarray:
    out, _ = run_kernel(trace=False, **inputs)
    return out


# revision 11
# speedup vs baseline: 2.5767x; 1.0858x over previous
"""Bass/Trainium2 kernel for nn_DWAMiddleLayer (low-rank MoE weight-assembly layer).

Math (reference):
    U    = pool[:, :1024].reshape(N, DB, R)      # [512, 256, 4]
    V    = pool[:, 1024:2048].reshape(N, R, DA)  # [512, 4, 256]
    bE   = pool[:, 2048:2304]                    # [512, 256]
    h_t  = h_A @ W_base.T
           + sum_r (alpha * (h_A @ V_r.T)) @ U_r          # never materialize W_assembled
           + alpha @ bE + b_base
    y    = h_A + gamma * h_t ; out = LayerNorm(y) * ln_scale + ln_bias

Distribution: data-parallel over batch B=2048 across 8 cores (BS=256 rows each);
pool/W_base/vectors replicated. h_t is computed in transposed space (feature dim
on partitions, batch on the free dim) so that every matmul contraction dim lands
on partitions naturally; layout transposes are PE identity-matmuls in bf16.
All matmul operands are bf16 (the gamma=1e-2 residual scaling makes matmul
rounding error negligible in the output); pool chunks arrive as SWDGE cast-DMAs,
small operands arrive in one packed HWDGE load and are cast on DVE. The
residual + LayerNorm path uses the untransposed fp32 h_A directly.
"""

import numpy as np

B, N, D_A, D_B, R = 2048, 512, 256, 256, 4
NC_COUNT = 8
BS = B // NC_COUNT  # 256 batch rows per core
P = 128
LN_EPS = 1e-5
POOL_W = D_B * R + R * D_A + D_B  # 2304 used columns of pool_vectors
U_OFF, V_OFF, BE_OFF = 0, D_B * R, D_B * R + R * D_A

# packed "smalls" tensor layout (fp32 elements per partition)
PK_HA = 0  # [2, 256]
PK_AL = 512  # [2, 512]
PK_WB = 1536  # [2, 256]
PK_ID = 2048  # 128 bf16 = 64 fp32 words
PK_BB = 2112  # [256] on partition 0 only
PK_W = 2368
# epilogue constants tensor [P, 513]: lsc(256) lbi(256) gamma(1)
EP_W = 513

_cache = {}


def _build_nc():
    import concourse.mybir as mybir
    import concourse.tile as tile
    from concourse import bacc

    fp32 = mybir.dt.float32
    bf16 = mybir.dt.bfloat16

    nc = bacc.Bacc("TRN2", target_bir_lowering=False)

    # ---- DRAM I/O (per-core shard shapes) ----
    d_pk = nc.dram_tensor("packed", [P, PK_W], fp32, kind="ExternalInput")
    d_ep = nc.dram_tensor("epconst", [P, EP_W], fp32, kind="ExternalInput")
    d_UV = nc.dram_tensor("UVpool", [N, POOL_W], fp32, kind="ExternalInput")
    d_out = nc.dram_tensor("out", [BS, D_A], fp32, kind="ExternalOutput")

    with tile.TileContext(nc) as tc:
        with (
            tc.tile_pool(name="persist", bufs=1) as persist,
            tc.tile_pool(name="stage", bufs=4) as stage,
            tc.tile_pool(name="sm", bufs=3) as sm,
            tc.tile_pool(name="pp_tr", bufs=3, space="PSUM") as pp_tr,
            tc.tile_pool(name="pp_t", bufs=2, space="PSUM") as pp_t,
            tc.tile_pool(name="pp_acc", bufs=1, space="PSUM") as pp_acc,
        ):
            # ---------- tiny constants ----------
            eps_col = persist.tile([P, 1], fp32)
            nc.vector.memset(eps_col, LN_EPS)
            ones_row = persist.tile([1, BS], bf16)
            nc.vector.memset(ones_row, 1.0)
            # warm the ACT Sqrt table so the LN tail doesn't pay ACT_TABLE_LOAD
            warm = sm.tile([P, 1], fp32, tag="warm")
            nc.scalar.activation(
                warm, eps_col, mybir.ActivationFunctionType.Sqrt, bias=eps_col
            )

            # ---------- loads ----------
            # one packed HWDGE DMA for all small operands
            pk = persist.tile([P, PK_W], fp32)
            nc.sync.dma_start(pk, d_pk[:])
            hA_sb = pk[:, PK_HA : PK_HA + 512].rearrange("p (o a) -> p o a", o=2)
            ident_b = pk[:, PK_ID : PK_ID + 64].bitcast(bf16)
            bb_row = pk[0:1, PK_BB : PK_BB + 256]

            # pool chunks via SWDGE cast-DMA (fp32 HBM read -> bf16 SBUF write)
            UVc = [
                stage.tile([P, POOL_W], bf16, tag="uvc", name=f"UVc{o}")
                for o in range(4)
            ]
            for o in range(4):
                nc.gpsimd.dma_start(UVc[o], d_UV[o * P : (o + 1) * P, :])

            # epilogue constants (HWDGE, low priority)
            ep = persist.tile([P, EP_W], fp32)
            nc.sync.dma_start(ep, d_ep[:])
            lsc_row = ep[:, 0:256]
            lbi_row = ep[:, 256:512]
            gamma_col = ep[:, 512:513]

            # bf16 casts of the packed smalls (DVE)
            hA_bf = sm.tile([P, 2, D_A], bf16, tag="hAbf")
            nc.vector.tensor_copy(hA_bf, hA_sb)
            alpha_bf = sm.tile([P, 2, N], bf16, tag="albf")
            nc.vector.tensor_copy(
                alpha_bf, pk[:, PK_AL : PK_AL + 1024].rearrange("p (o n) -> p o n", o=2)
            )
            Wb_bf = sm.tile([P, 2, D_A], bf16, tag="wbbf")
            nc.vector.tensor_copy(
                Wb_bf, pk[:, PK_WB : PK_WB + 512].rearrange("p (o a) -> p o a", o=2)
            )
            bb_bf = persist.tile([1, D_B], bf16)
            nc.vector.tensor_copy(bb_bf, bb_row)

            # ---------- transposes of small operands (PE identity-matmul, bf16) ----------
            hAT_b = persist.tile([P, 2, BS], bf16)  # [p_a, a_chunk, b]
            for ach in range(2):
                ps = pp_tr.tile([P, 512], fp32, tag="tr")
                for bch in range(2):
                    nc.tensor.matmul(
                        ps[:, bch * P : (bch + 1) * P],
                        lhsT=hA_bf[:, bch, ach * P : (ach + 1) * P],
                        rhs=ident_b,
                        start=True,
                        stop=True,
                    )
                nc.any.tensor_copy(hAT_b[:, ach], ps[:, :BS])

            # alpha^T -> bf16 [p_n, n_chunk, b]
            alphaT_b = persist.tile([P, 4, BS], bf16)
            for och in range(4):
                ps = pp_tr.tile([P, 512], fp32, tag="tr")
                for bch in range(2):
                    nc.tensor.matmul(
                        ps[:, bch * P : (bch + 1) * P],
                        lhsT=alpha_bf[:, bch, och * P : (och + 1) * P],
                        rhs=ident_b,
                        start=True,
                        stop=True,
                    )
                nc.any.tensor_copy(alphaT_b[:, och], ps[:, :BS])

            # W_base^T -> bf16 [p_a, a_chunk, c]
            WbT_b = persist.tile([P, 2, D_B], bf16)
            for ach in range(2):
                ps = pp_tr.tile([P, 512], fp32, tag="tr")
                for cch in range(2):
                    nc.tensor.matmul(
                        ps[:, cch * P : (cch + 1) * P],
                        lhsT=Wb_bf[:, cch, ach * P : (ach + 1) * P],
                        rhs=ident_b,
                        start=True,
                        stop=True,
                    )
                nc.any.tensor_copy(WbT_b[:, ach], ps[:, :D_B])

            # ---------- h_t^T accumulator: 2 psum tiles [c_half, b] ----------
            htT = [
                pp_acc.tile([P, BS], fp32, tag=f"acc{ch}", name=f"htT{ch}")
                for ch in range(2)
            ]
            started = [False, False]

            def acc_mm(ch, lhsT, rhs, last=False):
                nc.tensor.matmul(
                    htT[ch],
                    lhsT=lhsT,
                    rhs=rhs,
                    start=(not started[ch]),
                    stop=last,
                    skip_group_check=True,
                )
                started[ch] = True

            # ---------- main pipeline over expert chunks (o = n//128) ----------
            # V layout per pool row: f = V_OFF + r*256 + a  (r-major)
            # U layout per pool row: f = c*4 + r            (c-major)
            VT_b = persist.tile([P, 2, 2048], bf16)  # [p_a, a_chunk, r*512+o*128+pn]
            U_bfr = persist.tile([P, 4, R, D_B], bf16)  # [p_n, o, r, c]

            for o in range(4):
                V_bf = UVc[o][:, V_OFF : V_OFF + R * D_A]
                # transpose V chunk: blocks (r, a_half) of [128n x 128a]
                for ach in range(2):
                    ps = pp_tr.tile([P, 512], fp32, tag="tr")
                    for r in range(4):
                        nc.tensor.matmul(
                            ps[:, r * P : (r + 1) * P],
                            lhsT=V_bf[:, r * D_A + ach * P : r * D_A + (ach + 1) * P],
                            rhs=ident_b,
                            start=True,
                            stop=True,
                        )
                    # scatter the 4 r-blocks into VT at [r*512 + o*128]
                    dst = VT_b[:, ach].rearrange("p (r q) -> p r q", r=4)[
                        :, :, o * P : (o + 1) * P
                    ]
                    nc.any.tensor_copy(dst, ps[:].rearrange("p (r q) -> p r q", r=4))

                # destride U chunk (c r) -> (r c) in bf16 on DVE
                nc.vector.tensor_copy(
                    U_bfr[:, o],
                    UVc[o][:, U_OFF : U_OFF + D_B * R].rearrange(
                        "p (c r) -> p r c", r=R
                    ),
                )

                for r in range(4):
                    # mm1: t_r^T[n_chunk, b] = V_r @ h_A^T (contract a)
                    t_ps = pp_t.tile([P, BS], fp32, tag="t")
                    for ach in range(2):
                        nc.tensor.matmul(
                            t_ps,
                            lhsT=VT_b[:, ach, r * 512 + o * P : r * 512 + (o + 1) * P],
                            rhs=hAT_b[:, ach],
                            start=(ach == 0),
                            stop=(ach == 1),
                        )
                    # s_r^T = alpha^T * t_r^T  (evict psum -> bf16 sbuf)
                    s_bf = sm.tile([P, BS], bf16, tag="s")
                    nc.vector.tensor_mul(s_bf, t_ps, alphaT_b[:, o])
                    # mm2: h_t^T += U_r^T-chunks @ s_r^T (contract n)
                    for ch in range(2):
                        acc_mm(ch, U_bfr[:, o, r, ch * P : (ch + 1) * P], s_bf)

                # bias-mm for this chunk: h_t^T += biasE^T @ alpha^T (contract n)
                bE_o = UVc[o][:, BE_OFF : BE_OFF + D_B]
                for ch in range(2):
                    acc_mm(
                        ch, bE_o[:, ch * P : (ch + 1) * P], alphaT_b[:, o], last=(o == 3)
                    )

                if o == 0:
                    # base-mm + b_base rank-1, folded in early (no DMA deps left)
                    for ch in range(2):
                        for ach in range(2):
                            acc_mm(
                                ch, WbT_b[:, ach, ch * P : (ch + 1) * P], hAT_b[:, ach]
                            )
                        acc_mm(ch, bb_bf[:, ch * P : (ch + 1) * P], ones_row)

            # ---------- epilogue: transpose h_t back, residual + LayerNorm in fp32 ----------
            htT_bf = sm.tile([P, 2, BS], bf16, tag="htTbf")
            for ch in range(2):
                nc.any.tensor_copy(htT_bf[:, ch], htT[ch])

            ht_ps = pp_tr.tile([P, 512], fp32, tag="tr", name="ht_ps")
            for bch in range(2):
                for jch in range(2):
                    nc.tensor.matmul(
                        ht_ps[:, bch * 256 + jch * P : bch * 256 + (jch + 1) * P],
                        lhsT=htT_bf[:, jch, bch * P : (bch + 1) * P],
                        rhs=ident_b,
                        start=True,
                        stop=True,
                        skip_group_check=True,
                    )

            out_sb = sm.tile([P, 2, D_A], fp32, tag="out")
            for bch in range(2):
                # y = h_A + gamma * h_t   (fp32 residual)
                y_sb = sm.tile([P, D_A], fp32, tag="y")
                nc.vector.scalar_tensor_tensor(
                    y_sb,
                    in0=ht_ps[:, bch * 256 : bch * 256 + D_A],
                    scalar=gamma_col,
                    in1=hA_sb[:, bch],
                    op0=mybir.AluOpType.mult,
                    op1=mybir.AluOpType.add,
                )
                stats = sm.tile([P, 6], fp32, tag="st")
                nc.vector.bn_stats(stats, y_sb)
                mv = sm.tile([P, 2], fp32, tag="mv")
                nc.vector.bn_aggr(mv, stats)
                # rstd = 1/sqrt(var + eps)
                rstd = sm.tile([P, 1], fp32, tag="rstd")
                nc.scalar.activation(
                    rstd, mv[:, 1:2], mybir.ActivationFunctionType.Sqrt, bias=eps_col
                )
                nc.vector.reciprocal(rstd, rstd)
                # (y - mu) * rstd
                nc.vector.tensor_scalar(
                    out_sb[:, bch],
                    y_sb,
                    scalar1=mv[:, 0:1],
                    scalar2=rstd,
                    op0=mybir.AluOpType.subtract,
                    op1=mybir.AluOpType.mult,
                )
                # * ln_scale + ln_bias
                nc.vector.tensor_mul(out_sb[:, bch], out_sb[:, bch], lsc_row)
                nc.vector.tensor_add(out_sb[:, bch], out_sb[:, bch], lbi_row)

            nc.sync.dma_start(d_out[:].rearrange("(o p) c -> p o c", p=P), out_sb)

    nc.compile()
    return nc


def _get_nc():
    if "nc" not in _cache:
        _cache["nc"] = _build_nc()
    return _cache["nc"]


def make_in_maps(**inputs):
    """Shard full inputs into 8 per-core input maps."""
    import ml_dtypes

    f32 = lambda x: np.ascontiguousarray(np.asarray(x), dtype=np.float32)
    h_A = f32(inputs["h_A"])
    alpha = f32(inputs["alpha"])
    pool = np.asarray(inputs["pool_vectors"], dtype=np.float32)
    UVpool = np.ascontiguousarray(pool[:, :POOL_W])
    W_base = f32(inputs["W_base"])
    b_base = f32(inputs["b_base"]).reshape(D_B)
    gamma = float(np.asarray(inputs["gamma"]).reshape(()))
    ln_scale = f32(inputs["ln_scale"]).reshape(D_A)
    ln_bias = f32(inputs["ln_bias"]).reshape(D_A)

    ident = np.eye(P, dtype=np.float32).astype(ml_dtypes.bfloat16)
    ident_as_f32 = ident.view(np.uint16).reshape(P, P).view(np.uint32)  # not used
    ident_words = np.ascontiguousarray(ident).view(np.float32)  # [P, 64]

    ep = np.empty((P, EP_W), np.float32)
    ep[:, 0:256] = ln_scale[None, :]
    ep[:, 256:512] = ln_bias[None, :]
    ep[:, 512] = gamma

    wb_pk = np.ascontiguousarray(W_base.reshape(2, P, D_A).transpose(1, 0, 2)).reshape(
        P, 512
    )

    in_maps = []
    for i in range(NC_COUNT):
        sl = slice(i * BS, (i + 1) * BS)
        pk = np.zeros((P, PK_W), np.float32)
        pk[:, PK_HA : PK_HA + 512] = (
            h_A[sl].reshape(2, P, D_A).transpose(1, 0, 2).reshape(P, 512)
        )
        pk[:, PK_AL : PK_AL + 1024] = (
            alpha[sl].reshape(2, P, N).transpose(1, 0, 2).reshape(P, 1024)
        )
        pk[:, PK_WB : PK_WB + 512] = wb_pk
        pk[:, PK_ID : PK_ID + 64] = ident_words
        pk[0, PK_BB : PK_BB + 256] = b_base
        in_maps.append(
            {
                "packed": pk,
                "epconst": ep,
                "UVpool": UVpool,
            }
        )
    return in_maps


def run_kernel(trace=False, **inputs):
    from concourse.bass_utils import run_bass_kernel_spmd

    nc = _get_nc()
    in_maps = make_in_maps(**inputs)
    res = run_bass_kernel_spmd(nc, in_maps, core_ids=list(range(NC_COUNT)), trace=trace)
    out = np.concatenate([r["out"] for r in res.results], axis=0)
    return out.astype(np.float32), res


def kernel(**inputs) -> np.ndarray:
    out, _ = run_kernel(trace=False, **inputs)
    return out


# revision 12
# speedup vs baseline: 2.9223x; 1.1341x over previous
"""Bass/Trainium2 kernel for nn_DWAMiddleLayer (low-rank MoE weight-assembly layer).

Math (reference):
    U    = pool[:, :1024].reshape(N, DB, R)      # [512, 256, 4]
    V    = pool[:, 1024:2048].reshape(N, R, DA)  # [512, 4, 256]
    bE   = pool[:, 2048:2304]                    # [512, 256]
    h_t  = h_A @ W_base.T
           + sum_r (alpha * (h_A @ V_r.T)) @ U_r          # never materialize W_assembled
           + alpha @ bE + b_base
    y    = h_A + gamma * h_t ; out = LayerNorm(y) * ln_scale + ln_bias

Distribution: data-parallel over batch B=2048 across 8 cores (BS=256 rows each);
pool/W_base/vectors replicated. h_t is computed in transposed space (feature dim
on partitions, batch on the free dim) so that every matmul contraction dim lands
on partitions naturally; layout transposes are PE identity-matmuls in bf16.
All matmul operands are bf16 (the gamma=1e-2 residual scaling makes matmul
rounding error negligible in the output); pool chunks arrive as SWDGE cast-DMAs,
small operands arrive in one packed HWDGE load and are cast on DVE. The
residual + LayerNorm path uses the untransposed fp32 h_A directly.
"""

import numpy as np

B, N, D_A, D_B, R = 2048, 512, 256, 256, 4
NC_COUNT = 8
BS = B // NC_COUNT  # 256 batch rows per core
P = 128
LN_EPS = 1e-5
POOL_W = D_B * R + R * D_A + D_B  # 2304 used columns of pool_vectors
U_OFF, V_OFF, BE_OFF = 0, D_B * R, D_B * R + R * D_A

# packed "smalls" tensor layout (fp32 elements per partition)
PK_HA = 0  # [2, 256]
PK_WB = 512  # [2, 256]
PK_ID = 1024  # 128 bf16 = 64 fp32 words
PK_BB = 1088  # [256] on partition 0 only
PK_W = 1344
# epilogue constants tensor [P, 513]: lsc(256) lbi(256) gamma(1)
EP_W = 513

_cache = {}


def _build_nc():
    import concourse.mybir as mybir
    import concourse.tile as tile
    from concourse import bacc

    fp32 = mybir.dt.float32
    bf16 = mybir.dt.bfloat16

    nc = bacc.Bacc("TRN2", target_bir_lowering=False)

    # ---- DRAM I/O (per-core shard shapes) ----
    d_pk = nc.dram_tensor("packed", [P, PK_W], fp32, kind="ExternalInput")
    d_al = nc.dram_tensor("alpha", [BS, N], fp32, kind="ExternalInput")
    d_ep = nc.dram_tensor("epconst", [P, EP_W], fp32, kind="ExternalInput")
    d_UV = nc.dram_tensor("UVpool", [N, POOL_W], fp32, kind="ExternalInput")
    d_out = nc.dram_tensor("out", [BS, D_A], fp32, kind="ExternalOutput")

    with tile.TileContext(nc) as tc:
        with (
            tc.tile_pool(name="persist", bufs=1) as persist,
            tc.tile_pool(name="stage", bufs=4) as stage,
            tc.tile_pool(name="sm", bufs=3) as sm,
            tc.tile_pool(name="pp_tr", bufs=3, space="PSUM") as pp_tr,
            tc.tile_pool(name="pp_t", bufs=2, space="PSUM") as pp_t,
            tc.tile_pool(name="pp_acc", bufs=1, space="PSUM") as pp_acc,
        ):
            # ---------- tiny constants ----------
            eps_col = persist.tile([P, 1], fp32)
            nc.vector.memset(eps_col, LN_EPS)
            ones_row = persist.tile([1, BS], bf16)
            nc.vector.memset(ones_row, 1.0)
            # warm the ACT Sqrt table so the LN tail doesn't pay ACT_TABLE_LOAD
            warm = sm.tile([P, 1], fp32, tag="warm")
            nc.scalar.activation(
                warm, eps_col, mybir.ActivationFunctionType.Sqrt, bias=eps_col
            )

            # ---------- loads ----------
            # small packed HWDGE DMA (lands first; sync queue otherwise idle)
            pk = persist.tile([P, PK_W], fp32)
            nc.sync.dma_start(pk, d_pk[:])
            hA_sb = pk[:, PK_HA : PK_HA + 512].rearrange("p (o a) -> p o a", o=2)
            ident_b = pk[:, PK_ID : PK_ID + 64].bitcast(bf16)
            bb_row = pk[0:1, PK_BB : PK_BB + 256]

            # alpha via SWDGE cast-DMA, ahead of the pool chunks
            alpha_bf = persist.tile([P, 2, N], bf16)
            nc.gpsimd.dma_start(
                alpha_bf, d_al[:].rearrange("(o p) n -> p o n", p=P)
            )
            # pool chunks via SWDGE cast-DMA (fp32 HBM read -> bf16 SBUF write)
            UVc = [
                stage.tile([P, POOL_W], bf16, tag="uvc", name=f"UVc{o}")
                for o in range(4)
            ]
            for o in range(4):
                nc.gpsimd.dma_start(UVc[o], d_UV[o * P : (o + 1) * P, :])

            # epilogue constants (HWDGE, after the packed smalls)
            ep = persist.tile([P, EP_W], fp32)
            nc.sync.dma_start(ep, d_ep[:])
            lsc_row = ep[:, 0:256]
            lbi_row = ep[:, 256:512]
            gamma_col = ep[:, 512:513]

            # bf16 casts of the packed smalls (DVE)
            hA_bf = sm.tile([P, 2, D_A], bf16, tag="hAbf")
            nc.vector.tensor_copy(hA_bf, hA_sb)
            Wb_bf = sm.tile([P, 2, D_A], bf16, tag="wbbf")
            nc.vector.tensor_copy(
                Wb_bf, pk[:, PK_WB : PK_WB + 512].rearrange("p (o a) -> p o a", o=2)
            )
            bb_bf = persist.tile([1, D_B], bf16)
            nc.vector.tensor_copy(bb_bf, bb_row)

            # ---------- transposes of small operands (PE identity-matmul, bf16) ----------
            hAT_b = persist.tile([P, 2, BS], bf16)  # [p_a, a_chunk, b]
            for ach in range(2):
                ps = pp_tr.tile([P, 512], fp32, tag="tr")
                for bch in range(2):
                    nc.tensor.matmul(
                        ps[:, bch * P : (bch + 1) * P],
                        lhsT=hA_bf[:, bch, ach * P : (ach + 1) * P],
                        rhs=ident_b,
                        start=True,
                        stop=True,
                    )
                nc.any.tensor_copy(hAT_b[:, ach], ps[:, :BS])

            # alpha^T -> bf16 [p_n, n_chunk, b]
            alphaT_b = persist.tile([P, 4, BS], bf16)
            for och in range(4):
                ps = pp_tr.tile([P, 512], fp32, tag="tr")
                for bch in range(2):
                    nc.tensor.matmul(
                        ps[:, bch * P : (bch + 1) * P],
                        lhsT=alpha_bf[:, bch, och * P : (och + 1) * P],
                        rhs=ident_b,
                        start=True,
                        stop=True,
                    )
                nc.any.tensor_copy(alphaT_b[:, och], ps[:, :BS])

            # W_base^T -> bf16 [p_a, a_chunk, c]
            WbT_b = persist.tile([P, 2, D_B], bf16)
            for ach in range(2):
                ps = pp_tr.tile([P, 512], fp32, tag="tr")
                for cch in range(2):
                    nc.tensor.matmul(
                        ps[:, cch * P : (cch + 1) * P],
                        lhsT=Wb_bf[:, cch, ach * P : (ach + 1) * P],
                        rhs=ident_b,
                        start=True,
                        stop=True,
                    )
                nc.any.tensor_copy(WbT_b[:, ach], ps[:, :D_B])

            # ---------- h_t^T accumulator: 2 psum tiles [c_half, b] ----------
            htT = [
                pp_acc.tile([P, BS], fp32, tag=f"acc{ch}", name=f"htT{ch}")
                for ch in range(2)
            ]
            started = [False, False]

            def acc_mm(ch, lhsT, rhs, last=False):
                nc.tensor.matmul(
                    htT[ch],
                    lhsT=lhsT,
                    rhs=rhs,
                    start=(not started[ch]),
                    stop=last,
                    skip_group_check=True,
                )
                started[ch] = True

            # ---------- main pipeline over expert chunks (o = n//128) ----------
            # V layout per pool row: f = V_OFF + r*256 + a  (r-major)
            # U layout per pool row: f = c*4 + r            (c-major)
            VT_b = persist.tile([P, 2, 2048], bf16)  # [p_a, a_chunk, r*512+o*128+pn]
            U_bfr = persist.tile([P, 4, R, D_B], bf16)  # [p_n, o, r, c]

            for o in range(4):
                V_bf = UVc[o][:, V_OFF : V_OFF + R * D_A]
                # transpose V chunk: blocks (r, a_half) of [128n x 128a]
                for ach in range(2):
                    ps = pp_tr.tile([P, 512], fp32, tag="tr")
                    for r in range(4):
                        nc.tensor.matmul(
                            ps[:, r * P : (r + 1) * P],
                            lhsT=V_bf[:, r * D_A + ach * P : r * D_A + (ach + 1) * P],
                            rhs=ident_b,
                            start=True,
                            stop=True,
                        )
                    # scatter the 4 r-blocks into VT at [r*512 + o*128]
                    dst = VT_b[:, ach].rearrange("p (r q) -> p r q", r=4)[
                        :, :, o * P : (o + 1) * P
                    ]
                    nc.any.tensor_copy(dst, ps[:].rearrange("p (r q) -> p r q", r=4))

                # destride U chunk (c r) -> (r c) in bf16 on DVE
                nc.vector.tensor_copy(
                    U_bfr[:, o],
                    UVc[o][:, U_OFF : U_OFF + D_B * R].rearrange(
                        "p (c r) -> p r c", r=R
                    ),
                )

                for rp in range(2):
                    # mm1 for an r-pair: t_r^T[n_chunk, b] = V_r @ h_A^T (contract a)
                    t_ps = pp_t.tile([P, 2, BS], fp32, tag="t")
                    for rr in range(2):
                        r = rp * 2 + rr
                        for ach in range(2):
                            nc.tensor.matmul(
                                t_ps[:, rr],
                                lhsT=VT_b[
                                    :, ach, r * 512 + o * P : r * 512 + (o + 1) * P
                                ],
                                rhs=hAT_b[:, ach],
                                start=(ach == 0),
                                stop=(ach == 1),
                            )
                    # s_r^T = alpha^T * t_r^T for both r's in one DVE op
                    s_bf = sm.tile([P, 2, BS], bf16, tag="s")
                    nc.vector.tensor_mul(
                        s_bf, t_ps, alphaT_b[:, o : o + 1, :].to_broadcast((P, 2, BS))
                    )
                    # mm2: h_t^T += U_r^T-chunks @ s_r^T (contract n)
                    for rr in range(2):
                        r = rp * 2 + rr
                        for ch in range(2):
                            acc_mm(
                                ch, U_bfr[:, o, r, ch * P : (ch + 1) * P], s_bf[:, rr]
                            )

                # bias-mm for this chunk: h_t^T += biasE^T @ alpha^T (contract n)
                bE_o = UVc[o][:, BE_OFF : BE_OFF + D_B]
                for ch in range(2):
                    acc_mm(
                        ch, bE_o[:, ch * P : (ch + 1) * P], alphaT_b[:, o], last=(o == 3)
                    )

                if o == 0:
                    # base-mm + b_base rank-1, folded in early (no DMA deps left)
                    for ch in range(2):
                        for ach in range(2):
                            acc_mm(
                                ch, WbT_b[:, ach, ch * P : (ch + 1) * P], hAT_b[:, ach]
                            )
                        acc_mm(ch, bb_bf[:, ch * P : (ch + 1) * P], ones_row)

            # ---------- epilogue: transpose h_t back, residual + LayerNorm in fp32 ----------
            htT_bf = sm.tile([P, 2, BS], bf16, tag="htTbf")
            for ch in range(2):
                nc.any.tensor_copy(htT_bf[:, ch], htT[ch])

            ht_ps = pp_tr.tile([P, 512], fp32, tag="tr", name="ht_ps")
            for bch in range(2):
                for jch in range(2):
                    nc.tensor.matmul(
                        ht_ps[:, bch * 256 + jch * P : bch * 256 + (jch + 1) * P],
                        lhsT=htT_bf[:, jch, bch * P : (bch + 1) * P],
                        rhs=ident_b,
                        start=True,
                        stop=True,
                        skip_group_check=True,
                    )

            out_sb = sm.tile([P, 2, D_A], fp32, tag="out")
            for bch in range(2):
                # y = h_A + gamma * h_t   (fp32 residual)
                y_sb = sm.tile([P, D_A], fp32, tag="y")
                nc.vector.scalar_tensor_tensor(
                    y_sb,
                    in0=ht_ps[:, bch * 256 : bch * 256 + D_A],
                    scalar=gamma_col,
                    in1=hA_sb[:, bch],
                    op0=mybir.AluOpType.mult,
                    op1=mybir.AluOpType.add,
                )
                stats = sm.tile([P, 6], fp32, tag="st")
                nc.vector.bn_stats(stats, y_sb)
                mv = sm.tile([P, 2], fp32, tag="mv")
                nc.vector.bn_aggr(mv, stats)
                # rstd = 1/sqrt(var + eps)
                rstd = sm.tile([P, 1], fp32, tag="rstd")
                nc.scalar.activation(
                    rstd, mv[:, 1:2], mybir.ActivationFunctionType.Sqrt, bias=eps_col
                )
                nc.vector.reciprocal(rstd, rstd)
                # (y - mu) * rstd
                nc.vector.tensor_scalar(
                    out_sb[:, bch],
                    y_sb,
                    scalar1=mv[:, 0:1],
                    scalar2=rstd,
                    op0=mybir.AluOpType.subtract,
                    op1=mybir.AluOpType.mult,
                )
                # * ln_scale + ln_bias
                nc.vector.tensor_mul(out_sb[:, bch], out_sb[:, bch], lsc_row)
                nc.vector.tensor_add(out_sb[:, bch], out_sb[:, bch], lbi_row)
                nc.sync.dma_start(
                    d_out[bch * P : (bch + 1) * P, :], out_sb[:, bch]
                )

    nc.compile()
    return nc


def _get_nc():
    if "nc" not in _cache:
        _cache["nc"] = _build_nc()
    return _cache["nc"]


def make_in_maps(**inputs):
    """Shard full inputs into 8 per-core input maps."""
    import ml_dtypes

    f32 = lambda x: np.ascontiguousarray(np.asarray(x), dtype=np.float32)
    h_A = f32(inputs["h_A"])
    alpha = f32(inputs["alpha"])
    pool = np.asarray(inputs["pool_vectors"], dtype=np.float32)
    UVpool = np.ascontiguousarray(pool[:, :POOL_W])
    W_base = f32(inputs["W_base"])
    b_base = f32(inputs["b_base"]).reshape(D_B)
    gamma = float(np.asarray(inputs["gamma"]).reshape(()))
    ln_scale = f32(inputs["ln_scale"]).reshape(D_A)
    ln_bias = f32(inputs["ln_bias"]).reshape(D_A)

    ident = np.eye(P, dtype=np.float32).astype(ml_dtypes.bfloat16)
    ident_words = np.ascontiguousarray(ident).view(np.float32)  # [P, 64]

    ep = np.empty((P, EP_W), np.float32)
    ep[:, 0:256] = ln_scale[None, :]
    ep[:, 256:512] = ln_bias[None, :]
    ep[:, 512] = gamma

    wb_pk = np.ascontiguousarray(W_base.reshape(2, P, D_A).transpose(1, 0, 2)).reshape(
        P, 512
    )

    in_maps = []
    for i in range(NC_COUNT):
        sl = slice(i * BS, (i + 1) * BS)
        pk = np.zeros((P, PK_W), np.float32)
        pk[:, PK_HA : PK_HA + 512] = (
            h_A[sl].reshape(2, P, D_A).transpose(1, 0, 2).reshape(P, 512)
        )
        pk[:, PK_WB : PK_WB + 512] = wb_pk
        pk[:, PK_ID : PK_ID + 64] = ident_words
        pk[0, PK_BB : PK_BB + 256] = b_base
        in_maps.append(
            {
                "packed": pk,
                "alpha": np.ascontiguousarray(alpha[sl]),
                "epconst": ep,
                "UVpool": UVpool,
            }
        )
    return in_maps


def run_kernel(trace=False, **inputs):
    from concourse.bass_utils import run_bass_kernel_spmd

    nc = _get_nc()
    in_maps = make_in_maps(**inputs)
    res = run_bass_kernel_spmd(nc, in_maps, core_ids=list(range(NC_COUNT)), trace=trace)
    out = np.concatenate([r["out"] for r in res.results], axis=0)
    return out.astype(np.float32), res


def kernel(**inputs) -> np.ndarray:
    out, _ = run_kernel(trace=False, **inputs)
    return out
